# revision 1
# baseline (speedup 1.0000x reference)
"""GAT classifier Trainium kernel: host prep + Bass/Tile device kernel.

Sharding: dst-sharded nodes (8 slabs). Each core owns its node slab and all
edges whose dst is in the slab. Per layer: dense z=h@W on own slab, AllGather
z across cores, dma_gather z[src] rows per edge, segment softmax via
one-hot-matmul scatter into PSUM windows (W nodes per window), normalize,
relu -> h. Graph readout partials per core, AllReduce at the end, classifier
replicated.
"""
import sys
sys.path.insert(0, '/opt/trn_rl_repo')
sys.path.insert(0, '/root/.axon_site')

import numpy as np
import ml_dtypes
from contextlib import ExitStack

import concourse.bass as bass
import concourse.bacc as bacc
import concourse.tile as tile
import concourse.mybir as mybir
from concourse.bass_utils import run_bass_kernel_spmd

import concourse.tile as tile
import concourse.mybir as mybir
from concourse.tile import ScopedClock

MAX_WAITS = 1

class TileContextSafe(tile.TileContext):
    """TileContext whose tail drain splits sem waits across multiple
    instructions (walrus CTRL_NO struct only supports 2 sync waits)."""
    def _drain_and_barrier(self, tick_clock, wait_clock):
        drain_inst = self.nc.sync.drain()
        wait_clock.add_sem_waits(
            drain_inst.ins, ScopedClock({None: tick_clock.global_clock})
        )
        si = drain_inst.ins.sync_info
        waits = list(si.on_wait) if si is not None and si.on_wait else []
        if len(waits) > MAX_WAITS:
            drain_inst.ins.sync_info = mybir.SyncInfo(
                on_wait=waits[:MAX_WAITS],
                on_update=list(si.on_update) if si.on_update else [],
            )
            rest = waits[MAX_WAITS:]
            for i in range(0, len(rest), MAX_WAITS):
                extra = self.nc.sync.drain()
                extra.ins.sync_info = mybir.SyncInfo(
                    on_wait=rest[i:i + MAX_WAITS], on_update=[]
                )
        self.nc.all_engine_barrier()
        assert self.sems is not None
        popped = self.nc._tile_sem_poison_stack.pop()
        assert popped is self._sem_poison
        self.nc.clear_and_free_semaphores(list(self.sems.allocated().values()))
        self.nc.all_engine_barrier()


F32 = mybir.dt.float32
BF16 = mybir.dt.bfloat16
I16 = mybir.dt.int16
AF = mybir.ActivationFunctionType
ALU = mybir.AluOpType

NCORE = 8
WIN = 128         # nodes per scatter window (<=128)
GBLK = 128        # nodes per readout block
NG = 64           # graphs
LO_LIM = 32768    # int16 gather index limit


def _wrap16(vals):
    """dma_gather index layout: [128, n/16] int16, idx i -> [i%16, i//16],
    16-partition pattern replicated to 128."""
    v = np.asarray(vals, dtype=np.int16)
    assert len(v) % 16 == 0
    w = v.reshape(-1, 16).T  # [16, n/16]
    return np.tile(w, (8, 1))


def host_prep(x, e_feat, src, dst, gid, n_nodes=None):
    """Build per-core arrays + compile-time template."""
    N = x.shape[0] if n_nodes is None else n_nodes
    E = src.shape[0]
    assert N % NCORE == 0
    SLAB = N // NCORE
    n_win = (SLAB + WIN - 1) // WIN
    n_gblk = (SLAB + GBLK - 1) // GBLK

    SLABP = n_gblk * GBLK
    tmpl_np = NCORE * SLABP
    row_of = (src // SLAB) * SLABP + src % SLAB  # padded z_arr row per src
    core_of = dst // SLAB
    # per (core, window) edge lists, split by src < LO_LIM
    edge_lists = [[None] * n_win for _ in range(NCORE)]
    for k in range(NCORE):
        ek = np.nonzero(core_of == k)[0]
        d_loc = dst[ek] - k * SLAB
        order = np.argsort(d_loc, kind='stable')
        ek = ek[order]
        d_loc = d_loc[order]
        w_of = d_loc // WIN
        # boundaries per window
        starts = np.searchsorted(w_of, np.arange(n_win))
        ends = np.searchsorted(w_of, np.arange(1, n_win + 1))
        for w in range(n_win):
            e_w = ek[starts[w]:ends[w]]
            lo = e_w[src[e_w] < LO_LIM]
            hi = e_w[src[e_w] >= LO_LIM]
            edge_lists[k][w] = (lo, hi)

    # template: blocks per (window, lo/hi) = max over cores
    Lw = [max(-(-len(edge_lists[k][w][0]) // 128) for k in range(NCORE)) for w in range(n_win)]
    Hw = [max(-(-len(edge_lists[k][w][1]) // 128) for k in range(NCORE)) for w in range(n_win)]
    # every window needs >=1 block so psum is written before normalize
    for w in range(n_win):
        if Lw[w] + Hw[w] == 0:
            Lw[w] = 1
    GRP = 3
    groups = []        # (blk0, nlo, nhi, w0, nw)
    wblk = [None] * n_win   # per window: (lo block ids, hi block ids)
    b = 0
    for g0 in range(0, n_win, GRP):
        ws = range(g0, min(g0 + GRP, n_win))
        nlo = sum(Lw[w] for w in ws)
        nhi = sum(Hw[w] for w in ws)
        o = b
        for w in ws:
            wblk[w] = (list(range(o, o + Lw[w])), None)
            o += Lw[w]
        for w in ws:
            wblk[w] = (wblk[w][0], list(range(o, o + Hw[w])))
            o += Hw[w]
        groups.append((b, nlo, nhi, g0, len(ws)))
        b += nlo + nhi
    NB = b
    blocks = [None] * NB   # block id -> (window, is_hi)
    for w in range(n_win):
        for i in wblk[w][0]:
            blocks[i] = (w, 0)
        for i in wblk[w][1]:
            blocks[i] = (w, 1)

    assert n_win * WIN == n_gblk * GBLK
    assert max(nlo + nhi for _, nlo, nhi, _, _ in groups) <= 64
    tmpl = dict(N=N, E=E, SLAB=SLAB, SLABP=SLABP, n_win=n_win, n_gblk=n_gblk,
                NB=NB, Lw=Lw, Hw=Hw, blocks=blocks, groups=groups, wblk=wblk)

    per_core = []
    ef32 = np.asarray(e_feat, np.float32)
    for k in range(NCORE):
        gi = np.zeros(NB * 128, np.int16)      # z gather idx (pre-offset)
        zdi = np.zeros(NB * 128, np.int16)     # zd gather idx (slab-local)
        drel = np.full((128, NB), -1.0, np.float32)
        efp = np.zeros((128, NB, 16), ml_dtypes.bfloat16)
        for w in range(n_win):
            lo, hi = edge_lists[k][w]
            for is_hi, lst, ids in ((0, lo, wblk[w][0]), (1, hi, wblk[w][1])):
                if not ids:
                    continue
                sl = np.arange(len(lst))
                p = sl % 128
                bb = np.asarray(ids)[sl // 128]
                flat = bb * 128 + p
                gi[flat] = (row_of[lst] - (LO_LIM if is_hi else 0)).astype(np.int16)
                zdi[flat] = (dst[lst] - k * SLAB).astype(np.int16)
                drel[p, bb] = (dst[lst] - k * SLAB - w * WIN).astype(np.float32)
                efp[p, bb, :] = ef32[lst].astype(ml_dtypes.bfloat16)
        # wrap idx per call (concatenated in call order = block order, so one
        # big wrap works: call c covers idx range [blk0*128, (blk0+n)*128)
        d = dict(
            gi16=np.ascontiguousarray(_wrap16(gi)),
            zdi16=np.ascontiguousarray(_wrap16(zdi)),
            drel=np.ascontiguousarray(drel),
            efpT=np.ascontiguousarray(
                efp.transpose(2, 1, 0).reshape(16, -1)),
            grel=np.ascontiguousarray(
                np.pad(gid[k * SLAB:(k + 1) * SLAB].astype(np.float32),
                       (0, n_gblk * GBLK - SLAB), constant_values=-1.0)
                .reshape(n_gblk, GBLK).T),
            xT=None,  # filled by caller (own slab, feature-major bf16)
        )
        per_core.append(d)
    return tmpl, per_core


def build_kernel(tmpl, nb16_z, nb16_zd, sim_init=False):
    """Trace the SPMD program. nb16_*: #16-block chunks used for zd gather."""
    N, SLAB, SLABP = tmpl['N'], tmpl['SLAB'], tmpl['SLABP']
    n_win, n_gblk, NB = tmpl['n_win'], tmpl['n_gblk'], tmpl['NB']
    groups, wblk = tmpl['groups'], tmpl['wblk']
    Lw, Hw = tmpl['Lw'], tmpl['Hw']
    NP = NCORE * SLABP
    HI0 = min(LO_LIM, NP)

    nc = bacc.Bacc("TRN2", num_swdge_queues=2,
                   dynamic_dma_scratch_size=32768)
    P = lambda n, s, d: nc.declare_dram_parameter(n, s, d, isOutput=False)
    # per-core data
    gi16 = P("gi16", [128, NB * 8], I16)
    zdi16 = P("zdi16", [128, NB * 8], I16)
    drel = P("drel", [128, NB], F32)
    efpT = P("efpT", [16, NB * 128], BF16)
    ae3 = P("ae3", [16, 3], BF16)
    grel = P("grel", [128, n_gblk], F32)
    xT = P("xT", [128, n_gblk * GBLK], BF16)
    # replicated params
    Wl = P("Wl", [3, 128, 128], BF16)          # W_L feature-major
    WlT = P("WlT", [3, 128, 128], F32)         # W_L transposed (for w_d)
    adst = P("adst", [3, 128, 1], F32)
    asrcBC = P("asrcBC", [3, 128, 128], BF16)
    wattBC = P("wattBC", [128, 128], BF16)
    battBC = P("battBC", [128, 1], F32)
    iotaW = P("iotaW", [128, WIN], F32)
    iotaG = P("iotaG", [128, NG], F32)
    cntrec = P("cntrec", [NG, 1], F32)
    Wcls = P("Wcls", [3, 128, 10], F32)
    bcls = P("bcls", [1, 10], F32)
    eyeB = P("eyeB", [128, 128], BF16)
    out = nc.declare_dram_parameter("out", [NG, 10], F32, isOutput=True)

    # internal dram
    z_slab_d = nc.dram_tensor("z_slab_d", [SLABP, 128], BF16)
    z_arr = nc.dram_tensor("z_arr", [NP, 128], BF16, addr_space="Shared")
    zd_arr = nc.dram_tensor("zd_arr", [SLABP, 64], F32)
    ro_d = nc.dram_tensor("ro_d", [NG, 3 * 128], F32)
    ro_ar = nc.dram_tensor("ro_ar", [NG, 3 * 128], F32, addr_space="Shared")

    qctr = [0]

    def gq():
        q = qctr[0] & 1
        qctr[0] += 1
        return q

    def gather_blocks(zt_ap_fn, src_ap, idx_tile, b0, nblk):
        # split into <=32-block calls, alternating SWDGE queues
        o = 0
        while o < nblk:
            n = min(32, nblk - o)
            nc.gpsimd.dma_gather(
                zt_ap_fn(b0 + o, n), src_ap,
                idx_tile[:, (b0 + o) * 8:(b0 + o + n) * 8],
                n * 128, n * 128, src_ap.shape[-1],
                single_packet=False, queue_num=gq())
            o += n

    with ExitStack() as ctx:
        tc = ctx.enter_context(TileContextSafe(nc))
        const = ctx.enter_context(tc.tile_pool(name="const", bufs=1))
        dense = ctx.enter_context(tc.tile_pool(name="dense", bufs=1))
        gpool = ctx.enter_context(tc.tile_pool(name="gpool", bufs=3))
        efpool = ctx.enter_context(tc.tile_pool(name="efpool", bufs=2))
        zdp = ctx.enter_context(tc.tile_pool(name="zdp", bufs=2))
        spool = ctx.enter_context(tc.tile_pool(name="spool", bufs=6))
        psum = ctx.enter_context(tc.tile_pool(name="psum", bufs=3, space="PSUM"))
        psumd = ctx.enter_context(tc.tile_pool(name="psumd", bufs=2, space="PSUM"))
        psum1 = ctx.enter_context(tc.tile_pool(name="psum1", bufs=1, space="PSUM"))
        misc = ctx.enter_context(tc.tile_pool(name="misc", bufs=2))

        def load(pool, ap, shape, dtype, name):
            t = pool.tile(shape, dtype, tag=name)
            nc.sync.dma_start(t[:], ap)
            return t

        gi_t = load(const, gi16[:, :], [128, NB * 8], I16, "gi")
        zdi_t = load(const, zdi16[:, :], [128, NB * 8], I16, "zdi")
        drel_t = load(const, drel[:, :], [128, NB], F32, "drel")
        grel_t = load(const, grel[:, :], [128, n_gblk], F32, "grel")
        iW_t = load(const, iotaW[:, :], [128, WIN], F32, "iW")
        iG_t = load(const, iotaG[:, :], [128, NG], F32, "iG")
        watt_t = load(const, wattBC[:, :], [128, 128], BF16, "watt")
        batt_t = load(const, battBC[:, :], [128, 1], F32, "batt")
        eye_t = load(const, eyeB[:, :], [128, 128], BF16, "eye")
        ones_t = const.tile([128, 1], BF16, tag="ones")
        nc.vector.memset(ones_t[:], 1.0)
        cnt_t = load(const, cntrec[:, :], [NG, 1], F32, "cnt")
        bcls_t = load(const, bcls[:, :], [1, 10], F32, "bclst")

        ae3_t = load(const, ae3[:, :], [16, 3], BF16, "ae3")
        ec3_t = const.tile([128, NB, 3], F32, tag="ec3")

        hT = const.tile([128, n_gblk * GBLK], BF16, tag="hT")   # feature-major h
        nc.sync.dma_start(hT[:], xT[:, :])
        h_nm = const.tile([128, n_gblk, GBLK], BF16, tag="h_nm")  # node-major h
        ro_t = const.tile([NG, 3 * 128], F32, tag="ro")

        if sim_init:
            zinit = dense.tile([128, SLABP // 128 * 64], F32, tag="zsl")
            nc.vector.memset(zinit[:], 0.0)
            nc.sync.dma_start(zd_arr.rearrange("(c p) v -> p c v", p=128),
                              zinit[:].rearrange("p (c v) -> p c v", v=64))

        zd_t = const.tile([128, NB], F32, tag="zdv")
        ex_t = const.tile([128, NB], F32, tag="ex")

        for L in range(3):
            Wl_t = load(misc, Wl[L, :, :], [128, 128], BF16, "WlL")
            WlT_t = load(misc, WlT[L, :, :], [128, 128], F32, "WlTL")
            adst_t = load(misc, adst[L, :, :], [128, 1], F32, "adstL")
            asrc_t = load(misc, asrcBC[L, :, :], [128, 128], BF16, "asrcL")

            # w_d = W^T... out[f,1] = sum_k W[f,k] a_dst[k]: lhsT=WlT[k,f], rhs=adst[k]
            pwd = psum1.tile([128, 1], F32, tag="aux")
            nc.tensor.matmul(pwd[:], WlT_t[:], adst_t[:], start=True, stop=True)
            wd_t = misc.tile([128, 1], BF16, tag="wd")
            nc.vector.tensor_copy(wd_t[:], pwd[:])

            # layer>0: transpose own h (node-major) -> hT
            if L > 0:
                for c in range(n_gblk):
                    pt = psumd.tile([128, GBLK], BF16, tag="dz")
                    nc.tensor.transpose(pt[:], h_nm[:, c, :], eye_t[:])
                    nc.vector.tensor_copy(hT[:, c * GBLK:(c + 1) * GBLK], pt[:])

            # dense: z slab + zd slab
            zsl = dense.tile([128, n_gblk, 128], BF16, tag="zsl")
            zdc = dense.tile([128, n_gblk], F32, tag="zdc")
            for c in range(n_gblk):
                pz = psumd.tile([128, 129], F32, tag="dz")
                lhs = hT[:, c * GBLK:(c + 1) * GBLK]
                nc.tensor.matmul(pz[:, 0:128], lhs, Wl_t[:], start=True, stop=False)
                nc.tensor.matmul(pz[:, 128:129], lhs, wd_t[:], start=False, stop=True)
                nc.vector.tensor_copy(zsl[:, c, :], pz[:, 0:128])
                nc.vector.tensor_copy(zdc[:, c:c + 1], pz[:, 128:129])
            nc.sync.dma_start(
                z_slab_d.rearrange("(c p) f -> p c f", p=128), zsl[:, :, :])
            nc.sync.dma_start(
                zd_arr.rearrange("(c p) v -> p c v", p=128)[:, :, 0:1],
                zdc[:, :].rearrange("p (c o) -> p c o", o=1))

            cc = nc.gpsimd.collective_compute(
                "AllGather", ALU.bypass,
                ins=[z_slab_d[:, :]], outs=[z_arr[:, :]],
                replica_groups=[list(range(NCORE))])

            # zd gather (256B rows from zd_arr), chunked 16 blocks
            for c0 in range(0, NB, 32):
                nb = min(32, NB - c0)
                zg = zdp.tile([128, 32, 64], F32, tag="zg")
                nc.gpsimd.dma_gather(
                    zg[:, 0:nb, :], zd_arr[:, :],
                    zdi_t[:, c0 * 8:(c0 + nb) * 8], nb * 128, nb * 128, 64,
                    single_packet=False, queue_num=gq())
                nc.vector.tensor_copy(zd_t[:, c0:c0 + nb], zg[:, 0:nb, 0])

            if L == 0:
                # ec3 for all 3 layers via PE; overlaps the z AllGather wait
                for c0 in range(0, NB, 32):
                    nb = min(32, NB - c0)
                    eft = efpool.tile([16, 32 * 128], BF16, tag="eftT")
                    nc.sync.dma_start(eft[:, 0:nb * 128],
                                      efpT[:, c0 * 128:(c0 + nb) * 128])
                    for j0 in range(0, nb, 4):
                        n4 = min(4, nb - j0)
                        pec = psumd.tile([128, 12], F32, tag="dz")
                        for j in range(j0, j0 + n4):
                            o = (j - j0) * 3
                            nc.tensor.matmul(pec[:, o:o + 3],
                                             eft[:, j * 128:(j + 1) * 128],
                                             ae3_t[:], start=(j == j0),
                                             stop=(j == j0 + n4 - 1))
                        nc.vector.tensor_copy(
                            ec3_t[:, c0 + j0:c0 + j0 + n4, :],
                            pec[:, 0:n4 * 3])

            # per group: z gathers; per window: logits + scatter matmuls
            for (gb0, nlo, nhi, w0, nw) in groups:
                ng = nlo + nhi
                zt = gpool.tile([128, 64, 128], BF16, tag="zt")
                if nlo:
                    gather_blocks(lambda b, n: zt[:, b - gb0:b - gb0 + n, :],
                                  z_arr[:, :], gi_t, gb0, nlo)
                if nhi:
                    gather_blocks(
                        lambda b, n: zt[:, b - gb0:b - gb0 + n, :],
                        z_arr[HI0:NP, :], gi_t, gb0 + nlo, nhi)
                # per-block dots: zs
                for b in range(gb0, gb0 + ng):
                    i = b - gb0
                    scr = spool.tile([128, 128], BF16, tag="scr")
                    nc.vector.scalar_tensor_tensor(
                        out=scr[:], in0=zt[:, i, :], scalar=1.0,
                        in1=asrc_t[:], op0=ALU.mult, op1=ALU.mult,
                        accum_out=ex_t[:, b:b + 1])
                # logit = zs + zd + ec ; ex = exp(lrelu(logit, 0.2))
                sl_ = (slice(None), slice(gb0, gb0 + ng))
                nc.vector.tensor_tensor(out=ex_t[sl_], in0=ex_t[sl_],
                                        in1=zd_t[sl_], op=ALU.add)
                nc.vector.tensor_tensor(out=ex_t[sl_], in0=ex_t[sl_],
                                        in1=ec3_t[:, gb0:gb0 + ng, L],
                                        op=ALU.add)
                nc.vector.scalar_tensor_tensor(
                    out=ex_t[sl_], in0=ex_t[sl_], scalar=0.2,
                    in1=ex_t[sl_], op0=ALU.mult, op1=ALU.max)
                nc.scalar.activation(ex_t[sl_], ex_t[sl_], AF.Exp)

                for w in range(w0, w0 + nw):
                    ids = wblk[w][0] + wblk[w][1]
                    pw = psum.tile([WIN, 129], F32, tag="pw")
                    for j, b in enumerate(ids):
                        i = b - gb0
                        sp = spool.tile([128, WIN], BF16, tag="sp")
                        nc.vector.tensor_scalar(
                            out=sp[:], in0=iW_t[:], scalar1=drel_t[:, b:b + 1],
                            scalar2=ex_t[:, b:b + 1],
                            op0=ALU.is_equal, op1=ALU.mult)
                        nc.tensor.matmul(pw[:, 0:128], sp[:], zt[:, i, :],
                                         start=(j == 0), stop=False)
                        nc.tensor.matmul(pw[:, 128:129], sp[:], ones_t[:],
                                         start=False, stop=(j == len(ids) - 1))
                    # normalize + relu -> h node-major
                    rec = spool.tile([WIN, 1], F32, tag="rec")
                    den = spool.tile([WIN, 1], F32, tag="den")
                    nc.vector.tensor_scalar(out=den[:], in0=pw[:, 128:129],
                                            scalar1=1e-30, scalar2=None,
                                            op0=ALU.add)
                    nc.vector.reciprocal(rec[:], den[:])
                    hp = (w * WIN) % GBLK
                    hc = (w * WIN) // GBLK
                    nc.vector.tensor_scalar(
                        out=h_nm[hp:hp + WIN, hc, :], in0=pw[:, 0:128],
                        scalar1=rec[:], scalar2=0.0, op0=ALU.mult, op1=ALU.max)

            # readout partials for this layer
            att = misc.tile([128, n_gblk], F32, tag="att")
            pg = psum1.tile([NG, 128], F32, tag="pg")
            for c in range(n_gblk):
                scr = spool.tile([128, 128], F32, tag="scr")
                nc.vector.scalar_tensor_tensor(
                    out=scr[:], in0=h_nm[:, c, :], scalar=1.0,
                    in1=watt_t[:], op0=ALU.mult, op1=ALU.mult,
                    accum_out=att[:, c:c + 1])
            nc.vector.tensor_scalar(out=att[:], in0=att[:],
                                    scalar1=batt_t[:], scalar2=None,
                                    op0=ALU.add)
            nc.vector.scalar_tensor_tensor(
                out=att[:], in0=att[:], scalar=0.01, in1=att[:],
                op0=ALU.mult, op1=ALU.max)
            nc.scalar.activation(att[:], att[:], AF.Exp)
            for c in range(n_gblk):
                sg = spool.tile([128, NG], BF16, tag="sg")
                nc.vector.tensor_scalar(
                    out=sg[:], in0=iG_t[:], scalar1=grel_t[:, c:c + 1],
                    scalar2=att[:, c:c + 1], op0=ALU.is_equal, op1=ALU.mult)
                nc.tensor.matmul(pg[:], sg[:], h_nm[:, c, :],
                                 start=(c == 0), stop=(c == n_gblk - 1))
            nc.vector.tensor_copy(ro_t[:, L * 128:(L + 1) * 128], pg[:])

        # all-reduce readout partials, classifier, log_softmax
        nc.sync.dma_start(ro_d[:, :], ro_t[:])
        nc.gpsimd.collective_compute(
            "AllReduce", ALU.add, ins=[ro_d[:, :]], outs=[ro_ar[:, :]],
            replica_groups=[list(range(NCORE))])
        hg = misc.tile([NG, 3 * 128], F32, tag="hg")
        nc.sync.dma_start(hg[:], ro_ar[:, :])
        nc.vector.tensor_scalar(out=hg[:], in0=hg[:], scalar1=cnt_t[:],
                                scalar2=None, op0=ALU.mult)
        py_ = psum1.tile([NG, 10], F32, tag="py")
        eyeF = misc.tile([128, 128], F32, tag="eyeF")
        nc.vector.tensor_copy(eyeF[:], eye_t[:])
        for L in range(3):
            pt = psum1.tile([128, NG], F32, tag="aux")
            nc.tensor.transpose(pt[:], hg[:, L * 128:(L + 1) * 128], eyeF[0:NG, 0:NG])
            hgT = misc.tile([128, NG], F32, tag="hgT")
            nc.vector.tensor_copy(hgT[:], pt[:])
            Wc_t = load(misc, Wcls[L, :, :], [128, 10], F32, "WcL")
            nc.tensor.matmul(py_[:], hgT[:], Wc_t[:], start=(L == 0),
                             stop=False)
        onesr = misc.tile([1, NG], F32, tag="onesr")
        nc.vector.memset(onesr[:], 1.0)
        nc.tensor.matmul(py_[:], onesr[:], bcls_t[:], start=False, stop=True)
        y = misc.tile([NG, 10], F32, tag="y")
        m_ = misc.tile([NG, 1], F32, tag="m_")
        nc.vector.tensor_reduce(m_[:], py_[:], axis=mybir.AxisListType.X,
                                op=ALU.max)
        nc.vector.tensor_scalar(out=y[:], in0=py_[:], scalar1=m_[:],
                                scalar2=None, op0=ALU.subtract)
        e_ = misc.tile([NG, 10], F32, tag="e_")
        nc.scalar.activation(e_[:], y[:], AF.Exp)
        s_ = misc.tile([NG, 1], F32, tag="s_")
        nc.vector.tensor_reduce(s_[:], e_[:], axis=mybir.AxisListType.X,
                                op=ALU.add)
        nc.scalar.activation(s_[:], s_[:], AF.Ln)
        nc.vector.tensor_scalar(out=y[:], in0=y[:], scalar1=s_[:],
                                scalar2=None, op0=ALU.subtract)
        nc.sync.dma_start(out[:, :], y[:])
    nc.finalize()
    return nc


def make_inputs(tmpl, per_core, x, W1, a1, W2, a2, W3, a3,
                w_att, b_att, W_cls, b_cls, gid):
    N, SLAB, n_gblk = tmpl['N'], tmpl['SLAB'], tmpl['n_gblk']
    Ws = [np.asarray(w, np.float32) for w in (W1, W2, W3)]
    As = [np.asarray(a, np.float32) for a in (a1, a2, a3)]
    H = Ws[0].shape[1]
    cnt = np.bincount(gid, minlength=NG).astype(np.float32)
    rep = dict(
        Wl=np.stack([w.astype(ml_dtypes.bfloat16) for w in Ws]),
        WlT=np.stack([np.ascontiguousarray(w.T) for w in Ws]),
        adst=np.stack([a[H:2 * H].reshape(H, 1) for a in As]),
        asrcBC=np.stack([np.tile(a[None, :H], (128, 1)).astype(ml_dtypes.bfloat16) for a in As]),
        ae3=np.ascontiguousarray(
            np.stack([a[2 * H:] for a in As], axis=1).astype(ml_dtypes.bfloat16)),
        wattBC=np.tile(np.asarray(w_att, np.float32)[None, :], (128, 1)).astype(ml_dtypes.bfloat16),
        battBC=np.full((128, 1), np.float32(b_att)),
        iotaW=np.tile(np.arange(WIN, dtype=np.float32)[None, :], (128, 1)),
        iotaG=np.tile(np.arange(NG, dtype=np.float32)[None, :], (128, 1)),
        cntrec=(1.0 / np.maximum(cnt, 1.0)).reshape(NG, 1),
        Wcls=np.stack([np.asarray(W_cls, np.float32)[i * 128:(i + 1) * 128] for i in range(3)]),
        bcls=np.asarray(b_cls, np.float32).reshape(1, 10),
        eyeB=np.eye(128, dtype=ml_dtypes.bfloat16),
    )
    xf = np.asarray(x, np.float32)
    in_maps = []
    for k in range(NCORE):
        d = dict(per_core[k])
        d.pop('xT')
        xs = np.zeros((n_gblk * GBLK, 128), np.float32)
        xs[:SLAB] = xf[k * SLAB:(k + 1) * SLAB]
        d['xT'] = np.ascontiguousarray(xs.T).astype(ml_dtypes.bfloat16)
        d.update(rep)
        in_maps.append(d)
    return in_maps


LAST_EXEC_NS = None
TRACE = False
_CACHE = {}


def kernel(x, e_feat, src, dst, gid, W1, a1, W2, a2, W3, a3,
           w_att, b_att, W_cls, b_cls):
    global LAST_EXEC_NS
    x = np.asarray(x); e_feat = np.asarray(e_feat)
    src = np.asarray(src).astype(np.int64)
    dst = np.asarray(dst).astype(np.int64)
    gid = np.asarray(gid).astype(np.int64)
    tmpl, per_core = host_prep(x, e_feat, src, dst, gid)
    key = (tmpl['NB'], tuple(tmpl['Lw']), tuple(tmpl['Hw']))
    if key in _CACHE:
        nc = _CACHE[key]
    else:
        nc = build_kernel(tmpl, 0, 0)
        _CACHE[key] = nc
    in_maps = make_inputs(tmpl, per_core, x, W1, a1, W2, a2, W3, a3,
                          w_att, b_att, W_cls, b_cls, gid)
    res = run_bass_kernel_spmd(nc, in_maps, list(range(NCORE)), trace=TRACE)
    if res.exec_time_ns:
        LAST_EXEC_NS = res.exec_time_ns
    return np.asarray(res.results[0]["out"], dtype=np.float32)



# revision 8
# speedup vs baseline: 1.4876x; 1.4876x over previous
"""GAT classifier Trainium kernel: host prep + Bass/Tile device kernel.

Sharding: dst-sharded nodes (8 slabs). Each core owns its node slab and all
edges whose dst is in the slab. Per layer: dense z|zs|zd = h@[W|Wa_src|Wa_dst]
on own slab (node-major psum, cast+prev-layer-softmax-normalize on the way to
SBUF), AllGather the 130-col rows, expand locally into 512B-stride gather rows,
dma_gather [z|zs] per edge by src. Per-edge zd/1-over-den come from the dst
side via host-precomputed one-hot (transposed) matmuls on the PE. Scatter-sum
uses the one-hot as matmul rhs so h comes out FEATURE-major (no transposes for
the next dense phase); softmax normalization is deferred: h_raw = relu(U),
h = rec * h_raw applied per-node wherever h is consumed (relu commutes with
positive per-node scaling). Readout partials per core, AllReduce at the end,
classifier replicated.
"""
import sys
sys.path.insert(0, '/opt/trn_rl_repo')
sys.path.insert(0, '/root/.axon_site')

import numpy as np
import ml_dtypes
from contextlib import ExitStack

import concourse.bass as bass
import concourse.bacc as bacc
import concourse.tile as tile
import concourse.mybir as mybir
from concourse.bass_utils import run_bass_kernel_spmd
from concourse.tile import ScopedClock

MAX_WAITS = 1

class TileContextSafe(tile.TileContext):
    """TileContext whose tail drain splits sem waits across multiple
    instructions (walrus CTRL_NO struct only supports 2 sync waits)."""
    def _drain_and_barrier(self, tick_clock, wait_clock):
        drain_inst = self.nc.sync.drain()
        wait_clock.add_sem_waits(
            drain_inst.ins, ScopedClock({None: tick_clock.global_clock})
        )
        si = drain_inst.ins.sync_info
        waits = list(si.on_wait) if si is not None and si.on_wait else []
        if len(waits) > MAX_WAITS:
            drain_inst.ins.sync_info = mybir.SyncInfo(
                on_wait=waits[:MAX_WAITS],
                on_update=list(si.on_update) if si.on_update else [],
            )
            rest = waits[MAX_WAITS:]
            for i in range(0, len(rest), MAX_WAITS):
                extra = self.nc.sync.drain()
                extra.ins.sync_info = mybir.SyncInfo(
                    on_wait=rest[i:i + MAX_WAITS], on_update=[]
                )
        self.nc.all_engine_barrier()
        assert self.sems is not None
        popped = self.nc._tile_sem_poison_stack.pop()
        assert popped is self._sem_poison
        self.nc.clear_and_free_semaphores(list(self.sems.allocated().values()))
        self.nc.all_engine_barrier()


F32 = mybir.dt.float32
BF16 = mybir.dt.bfloat16
I16 = mybir.dt.int16
AF = mybir.ActivationFunctionType
ALU = mybir.AluOpType

NCORE = 8
WIN = 128         # nodes per scatter window (<=128)
NG = 64           # graphs
LO_LIM = 32768    # int16 gather index limit
ROW = 256         # bf16 elems per z_arr gather row (512B; 256B granularity)
NZ = 130          # payload cols: z(128) | zs | zd
GCALL = 32        # max blocks per dma_gather call


def _wrap16(vals):
    """dma_gather index layout: [128, n/16] int16, idx i -> [i%16, i//16],
    16-partition pattern replicated to 128."""
    v = np.asarray(vals, dtype=np.int16)
    assert len(v) % 16 == 0
    w = v.reshape(-1, 16).T  # [16, n/16]
    return np.tile(w, (8, 1))


def host_prep(x, e_feat, src, dst, gid, n_nodes=None):
    """Build per-core arrays + compile-time template."""
    N = x.shape[0] if n_nodes is None else n_nodes
    E = src.shape[0]
    assert N % NCORE == 0
    SLAB = N // NCORE
    n_win = (SLAB + WIN - 1) // WIN
    SLABP = n_win * WIN
    row_of = (src // SLAB) * SLABP + src % SLAB  # padded z_arr row per src
    core_of = dst // SLAB
    # per (core, window) edge lists, split by src row < LO_LIM
    edge_lists = [[None] * n_win for _ in range(NCORE)]
    for k in range(NCORE):
        ek = np.nonzero(core_of == k)[0]
        d_loc = dst[ek] - k * SLAB
        order = np.argsort(d_loc, kind='stable')
        ek = ek[order]
        d_loc = d_loc[order]
        w_of = d_loc // WIN
        starts = np.searchsorted(w_of, np.arange(n_win))
        ends = np.searchsorted(w_of, np.arange(1, n_win + 1))
        for w in range(n_win):
            e_w = ek[starts[w]:ends[w]]
            lo = e_w[row_of[e_w] < LO_LIM]
            hi = e_w[row_of[e_w] >= LO_LIM]
            edge_lists[k][w] = (lo, hi)

    # template: blocks per (window, lo/hi) = max over cores
    Lw = [max(-(-len(edge_lists[k][w][0]) // 128) for k in range(NCORE)) for w in range(n_win)]
    Hw = [max(-(-len(edge_lists[k][w][1]) // 128) for k in range(NCORE)) for w in range(n_win)]
    for w in range(n_win):
        if Lw[w] + Hw[w] == 0:
            Lw[w] = 1
    GRP = 2
    groups = []        # (blk0, nlo, nhi, w0, nw)
    wblk = [None] * n_win   # per window: (lo block ids, hi block ids)
    b = 0
    for g0 in range(0, n_win, GRP):
        ws = range(g0, min(g0 + GRP, n_win))
        nlo = sum(Lw[w] for w in ws)
        nhi = sum(Hw[w] for w in ws)
        o = b
        for w in ws:
            wblk[w] = (list(range(o, o + Lw[w])), None)
            o += Lw[w]
        for w in ws:
            wblk[w] = (wblk[w][0], list(range(o, o + Hw[w])))
            o += Hw[w]
        groups.append((b, nlo, nhi, g0, len(ws)))
        b += nlo + nhi
    NB = b
    blk_win = [None] * NB   # block id -> window
    for w in range(n_win):
        for i in wblk[w][0] + wblk[w][1]:
            blk_win[i] = w

    assert max(nlo + nhi for _, nlo, nhi, _, _ in groups) <= 64
    tmpl = dict(N=N, E=E, SLAB=SLAB, SLABP=SLABP, n_win=n_win,
                NB=NB, Lw=Lw, Hw=Hw, blk_win=blk_win, groups=groups, wblk=wblk)

    per_core = []
    ef32 = np.asarray(e_feat, np.float32)
    iota128 = np.arange(128, dtype=np.int32)
    for k in range(NCORE):
        gi = np.zeros(NB * 128, np.int16)               # z gather idx
        oh = np.zeros((128, NB, 128), ml_dtypes.bfloat16)   # [lane, blk, slot]
        ohT = np.zeros((128, NB, 128), ml_dtypes.bfloat16)  # [slot, blk, lane]
        efp = np.zeros((128, NB, 16), ml_dtypes.bfloat16)
        for w in range(n_win):
            lo, hi = edge_lists[k][w]
            for is_hi, lst, ids in ((0, lo, wblk[w][0]), (1, hi, wblk[w][1])):
                if not len(ids):
                    continue
                sl = np.arange(len(lst))
                p = sl % 128
                bb = np.asarray(ids)[sl // 128]
                gi[bb * 128 + p] = (row_of[lst] - (LO_LIM if is_hi else 0)).astype(np.int16)
                drel = (dst[lst] - k * SLAB - w * WIN).astype(np.int32)
                oh[p, bb, drel] = 1.0
                ohT[drel, bb, p] = 1.0
                efp[p, bb, :] = ef32[lst].astype(ml_dtypes.bfloat16)
        d = dict(
            gi16=np.ascontiguousarray(_wrap16(gi)),
            onehot=np.ascontiguousarray(oh.reshape(128, NB * 128)),
            onehotT=np.ascontiguousarray(ohT.reshape(128, NB * 128)),
            efpT=np.ascontiguousarray(
                efp.transpose(2, 1, 0).reshape(16, -1)),
            grel=np.ascontiguousarray(
                np.pad(gid[k * SLAB:(k + 1) * SLAB].astype(np.float32),
                       (0, SLABP - SLAB), constant_values=-1.0)
                .reshape(n_win, WIN).T),
            xT=None,  # filled by caller (own slab, feature-major bf16)
        )
        per_core.append(d)
    return tmpl, per_core


def build_kernel(tmpl, sim_init=False):
    """Trace the SPMD program."""
    N, SLAB, SLABP = tmpl['N'], tmpl['SLAB'], tmpl['SLABP']
    n_win, NB = tmpl['n_win'], tmpl['NB']
    groups, wblk, blk_win = tmpl['groups'], tmpl['wblk'], tmpl['blk_win']
    GMAX = max(nl + nh for _, nl, nh, _, _ in groups)
    NP = NCORE * SLABP
    HI0 = min(LO_LIM, NP)

    nc = bacc.Bacc("TRN2", num_swdge_queues=2,
                   dynamic_dma_scratch_size=32768)
    P = lambda n, s, d: nc.declare_dram_parameter(n, s, d, isOutput=False)
    # per-core data
    gi16 = P("gi16", [128, NB * 8], I16)
    onehot = P("onehot", [128, NB * 128], BF16)
    onehotT = P("onehotT", [128, NB * 128], BF16)
    efpT = P("efpT", [16, NB * 128], BF16)
    ae3 = P("ae3", [16, 3], BF16)
    grel = P("grel", [128, n_win], F32)
    xT = P("xT", [128, SLABP], BF16)
    # replicated params
    Wfull = P("Wfull", [3, 128, NZ], BF16)     # [W | W@a_src | W@a_dst]
    wattBC = P("wattBC", [128, 128], BF16)
    battBC = P("battBC", [128, 1], F32)
    iotaG = P("iotaG", [128, NG], F32)
    cntrec = P("cntrec", [NG, 1], F32)
    Wcls = P("Wcls", [3, 128, 10], F32)
    bcls = P("bcls", [1, 10], F32)
    eyeB = P("eyeB", [128, 128], BF16)
    out = nc.declare_dram_parameter("out", [NG, 10], F32, isOutput=True)

    # internal dram
    z_small_d = nc.dram_tensor("z_small_d", [SLABP, NZ], BF16)
    z_small_a = nc.dram_tensor("z_small_a", [NP, NZ], BF16, addr_space="Shared")
    z_arr = nc.dram_tensor("z_arr", [NP, ROW], BF16)
    ro_d = nc.dram_tensor("ro_d", [NG, 3 * 128], F32)
    ro_ar = nc.dram_tensor("ro_ar", [NG, 3 * 128], F32, addr_space="Shared")

    qctr = [0]

    def gq():
        q = qctr[0] & 1
        qctr[0] += 1
        return q

    def gather_blocks(zt, zoff, src_ap, idx_tile, b0, nblk, gb0):
        # split into <=GCALL-block calls, alternating SWDGE queues
        o = 0
        while o < nblk:
            n = min(GCALL, nblk - o)
            t0 = b0 + o - gb0
            nc.gpsimd.dma_gather(
                zt[:, t0:t0 + n, :], src_ap,
                idx_tile[:, (b0 + o) * 8:(b0 + o + n) * 8],
                n * 128, n * 128, ROW,
                single_packet=False, queue_num=gq())
            o += n

    with ExitStack() as ctx:
        tc = ctx.enter_context(TileContextSafe(nc))
        const = ctx.enter_context(tc.tile_pool(name="const", bufs=1))
        gpool = ctx.enter_context(tc.tile_pool(name="gpool", bufs=2))
        ohpool = ctx.enter_context(tc.tile_pool(name="ohpool", bufs=2))
        ohTpool = ctx.enter_context(tc.tile_pool(name="ohTpool", bufs=2))
        efpool = ctx.enter_context(tc.tile_pool(name="efpool", bufs=2))
        spool = ctx.enter_context(tc.tile_pool(name="spool", bufs=6))
        lgpool = ctx.enter_context(tc.tile_pool(name="lgpool", bufs=3))
        psA = ctx.enter_context(tc.tile_pool(name="psA", bufs=3, space="PSUM"))
        psB = ctx.enter_context(tc.tile_pool(name="psB", bufs=4, space="PSUM"))
        misc = ctx.enter_context(tc.tile_pool(name="misc", bufs=2))

        def load(pool, ap, shape, dtype, name):
            t = pool.tile(shape, dtype, tag=name)
            nc.sync.dma_start(t[:], ap)
            return t

        gi_t = load(const, gi16[:, :], [128, NB * 8], I16, "gi")
        grel_t = load(const, grel[:, :], [128, n_win], F32, "grel")
        iG_t = load(const, iotaG[:, :], [128, NG], F32, "iG")
        watt_t = load(const, wattBC[:, :], [128, 128], BF16, "watt")
        batt_t = load(const, battBC[:, :], [128, 1], F32, "batt")
        eye_t = load(const, eyeB[:, :], [128, 128], BF16, "eye")
        cnt_t = load(const, cntrec[:, :], [NG, 1], F32, "cnt")
        bcls_t = load(const, bcls[:, :], [1, 10], F32, "bclst")

        ae3_t = load(const, ae3[:, :], [16, 3], BF16, "ae3")
        ec3_t = const.tile([128, NB, 3], BF16, tag="ec3")

        hT = const.tile([128, SLABP], BF16, tag="hT")   # feature-major h
        nc.sync.dma_start(hT[:], xT[:, :])
        h_nm = const.tile([128, n_win, 128], BF16, tag="h_nm")  # node-major h
        ro_t = const.tile([NG, 3 * 128], F32, tag="ro")
        rec_t = const.tile([128, n_win], F32, tag="rec")  # per-node 1/den
        nc.vector.memset(rec_t[:], 1.0)
        rec16 = const.tile([128, n_win], BF16, tag="rec16")
        zrow = const.tile([128, n_win, NZ], BF16, tag="zrow")

        if sim_init:
            zinit = misc.tile([128, NP * ROW // 128], BF16, tag="zi")
            nc.vector.memset(zinit[:], 0.0)
            nc.sync.dma_start(
                z_arr.rearrange("(c p) v -> p (c v)", p=128), zinit[:])

        def readout(L):
            """Readout partials for layer L from hT (raw) + rec_t."""
            att = misc.tile([128, n_win], F32, tag="att")
            for c in range(n_win):
                pt = psA.tile([128, 128], BF16, tag="pA")
                nc.tensor.transpose(pt[:], hT[:, c * 128:(c + 1) * 128],
                                    eye_t[:])
                nc.vector.tensor_copy(h_nm[:, c, :], pt[:])
                scr = spool.tile([128, 128], BF16, tag="scr")
                nc.vector.scalar_tensor_tensor(
                    out=scr[:], in0=h_nm[:, c, :], scalar=1.0,
                    in1=watt_t[:], op0=ALU.mult, op1=ALU.mult,
                    accum_out=att[:, c:c + 1])
            nc.vector.tensor_scalar(out=att[:], in0=att[:],
                                    scalar1=batt_t[:], scalar2=None,
                                    op0=ALU.add)
            nc.vector.scalar_tensor_tensor(
                out=att[:], in0=att[:], scalar=0.01, in1=att[:],
                op0=ALU.mult, op1=ALU.max)
            nc.scalar.activation(att[:], att[:], AF.Exp)
            pg = psB.tile([NG, 128], F32, tag="pB")
            for c in range(n_win):
                sg = spool.tile([128, NG], BF16, tag="sg")
                nc.vector.tensor_scalar(
                    out=sg[:], in0=iG_t[:], scalar1=grel_t[:, c:c + 1],
                    scalar2=att[:, c:c + 1], op0=ALU.is_equal, op1=ALU.mult)
                nc.tensor.matmul(pg[:], sg[:], h_nm[:, c, :],
                                 start=(c == 0), stop=(c == n_win - 1))
            nc.vector.tensor_copy(ro_t[:, L * 128:(L + 1) * 128], pg[:])

        for L in range(3):
            Wf_t = load(misc, Wfull[L, :, :], [128, NZ], BF16, "WfL")

            # dense: z|zs|zd for own slab, node-major; fold prev-layer rec
            for c in range(n_win):
                pz = psA.tile([128, NZ], F32, tag="pA")
                nc.tensor.matmul(pz[:], hT[:, c * 128:(c + 1) * 128], Wf_t[:],
                                 start=True, stop=True)
                nc.vector.tensor_copy(zrow[:, c, :], pz[:])
            nc.sync.dma_start(
                z_small_d.rearrange("(c p) v -> p c v", p=128), zrow[:, :, :])

            nc.gpsimd.collective_compute(
                "AllGather", ALU.bypass,
                ins=[z_small_d[:, :]], outs=[z_small_a[:, :]],
                replica_groups=[list(range(NCORE))])

            if L == 0:
                # ec3 for all 3 layers via PE; overlaps the z AllGather
                for c0 in range(0, NB, 32):
                    nb = min(32, NB - c0)
                    eft = efpool.tile([16, 32 * 128], BF16, tag="eftT")
                    nc.sync.dma_start(eft[:, 0:nb * 128],
                                      efpT[:, c0 * 128:(c0 + nb) * 128])
                    for j0 in range(0, nb, 4):
                        n4 = min(4, nb - j0)
                        pec = psB.tile([128, 12], F32, tag="pB")
                        for j in range(j0, j0 + n4):
                            o = (j - j0) * 3
                            nc.tensor.matmul(pec[:, o:o + 3],
                                             eft[:, j * 128:(j + 1) * 128],
                                             ae3_t[:], start=(j == j0),
                                             stop=(j == j0 + n4 - 1))
                        nc.vector.tensor_copy(
                            ec3_t[:, c0 + j0:c0 + j0 + n4, :],
                            pec[:, 0:n4 * 3])
            else:
                readout(L - 1)   # overlaps the AllGather

            # expand 260B rows -> 512B-stride gather rows
            nc.sync.dma_start(z_arr[:, 0:NZ], z_small_a[:, :])

            # edge phase
            for (gb0, nlo, nhi, w0, nw) in groups:
                ng = nlo + nhi
                zt = gpool.tile([128, GMAX, ROW], BF16, tag="zt")
                if nlo:
                    gather_blocks(zt, 0, z_arr[:, :], gi_t, gb0, nlo, gb0)
                if nhi:
                    gather_blocks(zt, 0, z_arr[HI0:NP, :], gi_t,
                                  gb0 + nlo, nhi, gb0)
                oh_t = ohpool.tile([128, GMAX, 128], BF16, tag="oh")
                nc.sync.dma_start(oh_t[:, 0:ng, :],
                                  onehot[:, gb0 * 128:(gb0 + ng) * 128])
                ohT_t = ohTpool.tile([128, GMAX, 128], BF16, tag="ohT")
                nc.sync.dma_start(ohT_t[:, 0:ng, :],
                                  onehotT[:, gb0 * 128:(gb0 + ng) * 128])

                # per-edge zd via one-hot^T @ zd_win
                pzd = psB.tile([128, GMAX], F32, tag="pB")
                for j in range(ng):
                    w = blk_win[gb0 + j]
                    nc.tensor.matmul(pzd[:, j:j + 1], ohT_t[:, j, :],
                                     zrow[:, w, 129:130],
                                     start=True, stop=True)
                # logits + exp (per group)
                lg = lgpool.tile([128, GMAX], F32, tag="lg")
                sl = (slice(None), slice(0, ng))
                nc.vector.tensor_tensor(out=lg[sl], in0=zt[:, 0:ng, 128],
                                        in1=ec3_t[:, gb0:gb0 + ng, L],
                                        op=ALU.add)
                nc.vector.tensor_tensor(out=lg[sl], in0=lg[sl],
                                        in1=pzd[:, 0:ng], op=ALU.add)
                nc.vector.scalar_tensor_tensor(
                    out=lg[sl], in0=lg[sl], scalar=0.2,
                    in1=lg[sl], op0=ALU.mult, op1=ALU.max)
                ex32 = lgpool.tile([128, GMAX], F32, tag="ex32")
                nc.scalar.activation(ex32[sl], lg[sl], AF.Exp)
                ex16 = lgpool.tile([128, GMAX], BF16, tag="ex")
                nc.vector.tensor_copy(ex16[sl], ex32[sl])

                # denominators per window; rec = 1/(den+eps)
                pden = psB.tile([128, 4], F32, tag="pB")
                for wi, w in enumerate(range(w0, w0 + nw)):
                    ids = wblk[w][0] + wblk[w][1]
                    for jj, b in enumerate(ids):
                        nc.tensor.matmul(pden[:, wi:wi + 1],
                                         oh_t[:, b - gb0, :],
                                         ex16[:, b - gb0:b - gb0 + 1],
                                         start=(jj == 0),
                                         stop=(jj == len(ids) - 1))
                den = lgpool.tile([128, 4], F32, tag="den")
                nc.vector.tensor_scalar(out=den[:, 0:nw], in0=pden[:, 0:nw],
                                        scalar1=1e-30, scalar2=None,
                                        op0=ALU.add)
                nc.vector.reciprocal(rec_t[:, w0:w0 + nw], den[:, 0:nw])
                nc.vector.tensor_copy(rec16[:, w0:w0 + nw],
                                      rec_t[:, w0:w0 + nw])

                # per-edge rec via one-hot^T @ rec_win
                prec = psB.tile([128, GMAX], F32, tag="pB")
                for j in range(ng):
                    w = blk_win[gb0 + j]
                    nc.tensor.matmul(prec[:, j:j + 1], ohT_t[:, j, :],
                                     rec16[:, w:w + 1],
                                     start=True, stop=True)

                # scaled messages + scatter (feature-major out)
                for wi, w in enumerate(range(w0, w0 + nw)):
                    ids = wblk[w][0] + wblk[w][1]
                    pU = psA.tile([128, 128], F32, tag="pA")
                    for jj, b in enumerate(ids):
                        j = b - gb0
                        zte = spool.tile([128, 128], BF16, tag="zte")
                        nc.vector.tensor_scalar(
                            out=zte[:], in0=zt[:, j, 0:128],
                            scalar1=ex32[:, j:j + 1],
                            scalar2=prec[:, j:j + 1],
                            op0=ALU.mult, op1=ALU.mult)
                        nc.tensor.matmul(pU[:], zte[:], oh_t[:, j, :],
                                         start=(jj == 0),
                                         stop=(jj == len(ids) - 1))
                    # h_raw = relu(U), feature-major; normalization deferred
                    nc.vector.tensor_scalar(
                        out=hT[:, w * 128:(w + 1) * 128], in0=pU[:],
                        scalar1=0.0, scalar2=None, op0=ALU.max)

        readout(2)

        # all-reduce readout partials, classifier, log_softmax
        nc.sync.dma_start(ro_d[:, :], ro_t[:])
        nc.gpsimd.collective_compute(
            "AllReduce", ALU.add, ins=[ro_d[:, :]], outs=[ro_ar[:, :]],
            replica_groups=[list(range(NCORE))])
        hg = misc.tile([NG, 3 * 128], F32, tag="hg")
        nc.sync.dma_start(hg[:], ro_ar[:, :])
        nc.vector.tensor_scalar(out=hg[:], in0=hg[:], scalar1=cnt_t[:],
                                scalar2=None, op0=ALU.mult)
        py_ = psB.tile([NG, 10], F32, tag="pB")
        eyeF = misc.tile([128, 128], F32, tag="eyeF")
        nc.vector.tensor_copy(eyeF[:], eye_t[:])
        for L in range(3):
            pt = psA.tile([128, NG], F32, tag="pA")
            nc.tensor.transpose(pt[:], hg[:, L * 128:(L + 1) * 128],
                                eyeF[0:NG, 0:NG])
            hgT = misc.tile([128, NG], F32, tag="hgT")
            nc.vector.tensor_copy(hgT[:], pt[:])
            Wc_t = load(misc, Wcls[L, :, :], [128, 10], F32, "WcL")
            nc.tensor.matmul(py_[:], hgT[:], Wc_t[:], start=(L == 0),
                             stop=False)
        onesr = misc.tile([1, NG], F32, tag="onesr")
        nc.vector.memset(onesr[:], 1.0)
        nc.tensor.matmul(py_[:], onesr[:], bcls_t[:], start=False, stop=True)
        y = misc.tile([NG, 10], F32, tag="y")
        m_ = misc.tile([NG, 1], F32, tag="m_")
        nc.vector.tensor_reduce(m_[:], py_[:], axis=mybir.AxisListType.X,
                                op=ALU.max)
        nc.vector.tensor_scalar(out=y[:], in0=py_[:], scalar1=m_[:],
                                scalar2=None, op0=ALU.subtract)
        e_ = misc.tile([NG, 10], F32, tag="e_")
        nc.scalar.activation(e_[:], y[:], AF.Exp)
        s_ = misc.tile([NG, 1], F32, tag="s_")
        nc.vector.tensor_reduce(s_[:], e_[:], axis=mybir.AxisListType.X,
                                op=ALU.add)
        nc.scalar.activation(s_[:], s_[:], AF.Ln)
        nc.vector.tensor_scalar(out=y[:], in0=y[:], scalar1=s_[:],
                                scalar2=None, op0=ALU.subtract)
        nc.sync.dma_start(out[:, :], y[:])
    nc.finalize()
    return nc


def make_inputs(tmpl, per_core, x, W1, a1, W2, a2, W3, a3,
                w_att, b_att, W_cls, b_cls, gid):
    N, SLAB, n_win = tmpl['N'], tmpl['SLAB'], tmpl['n_win']
    Ws = [np.asarray(w, np.float32) for w in (W1, W2, W3)]
    As = [np.asarray(a, np.float32) for a in (a1, a2, a3)]
    H = Ws[0].shape[1]
    cnt = np.bincount(gid, minlength=NG).astype(np.float32)
    Wf = np.zeros((3, 128, NZ), np.float32)
    for i, (W, a) in enumerate(zip(Ws, As)):
        Wf[i, :, 0:128] = W
        Wf[i, :, 128] = W @ a[:H]
        Wf[i, :, 129] = W @ a[H:2 * H]
    rep = dict(
        Wfull=Wf.astype(ml_dtypes.bfloat16),
        ae3=np.ascontiguousarray(
            np.stack([a[2 * H:] for a in As], axis=1).astype(ml_dtypes.bfloat16)),
        wattBC=np.tile(np.asarray(w_att, np.float32)[None, :], (128, 1)).astype(ml_dtypes.bfloat16),
        battBC=np.full((128, 1), np.float32(b_att)),
        iotaG=np.tile(np.arange(NG, dtype=np.float32)[None, :], (128, 1)),
        cntrec=(1.0 / np.maximum(cnt, 1.0)).reshape(NG, 1),
        Wcls=np.stack([np.asarray(W_cls, np.float32)[i * 128:(i + 1) * 128] for i in range(3)]),
        bcls=np.asarray(b_cls, np.float32).reshape(1, 10),
        eyeB=np.eye(128, dtype=ml_dtypes.bfloat16),
    )
    xf = np.asarray(x, np.float32)
    in_maps = []
    for k in range(NCORE):
        d = dict(per_core[k])
        d.pop('xT')
        xs = np.zeros((tmpl['SLABP'], 128), np.float32)
        xs[:SLAB] = xf[k * SLAB:(k + 1) * SLAB]
        d['xT'] = np.ascontiguousarray(xs.T).astype(ml_dtypes.bfloat16)
        d.update(rep)
        in_maps.append(d)
    return in_maps


LAST_EXEC_NS = None
TRACE = False
_CACHE = {}


def kernel(x, e_feat, src, dst, gid, W1, a1, W2, a2, W3, a3,
           w_att, b_att, W_cls, b_cls):
    global LAST_EXEC_NS
    x = np.asarray(x); e_feat = np.asarray(e_feat)
    src = np.asarray(src).astype(np.int64)
    dst = np.asarray(dst).astype(np.int64)
    gid = np.asarray(gid).astype(np.int64)
    tmpl, per_core = host_prep(x, e_feat, src, dst, gid)
    key = (tmpl['NB'], tuple(tmpl['Lw']), tuple(tmpl['Hw']))
    if key in _CACHE:
        nc = _CACHE[key]
    else:
        nc = build_kernel(tmpl)
        _CACHE[key] = nc
    in_maps = make_inputs(tmpl, per_core, x, W1, a1, W2, a2, W3, a3,
                          w_att, b_att, W_cls, b_cls, gid)
    res = run_bass_kernel_spmd(nc, in_maps, list(range(NCORE)), trace=TRACE)
    if res.exec_time_ns:
        LAST_EXEC_NS = res.exec_time_ns
    return np.asarray(res.results[0]["out"], dtype=np.float32)
